# revision 6
# baseline (speedup 1.0000x reference)
"""1D horizontal correlation (FlowNet cost volume, kernel_size=1) on 8 TRN2 cores.

out[b, d+4, y, x] = mean_c x1[b,c,y,x] * x2[b,c,y,x+d],  d in [-4, 4], OOB -> 0

Strategy (per core = one batch element; data-parallel over B=8):
- Flatten (H, W) -> S=30720 positions, C=128 = partition dim.
- For each 128-position tile t, a TensorE band matmul (bf16 in, fp32 psum):
      psum[m, n] = sum_c x1[c, 128t+m] * x2[c, 128t-4+n],  n in [0, 136)
  holds the 9 needed outputs on diagonals psum[m, m+j], j=0..8.
- GRP=3 tiles go into one PSUM bank; BANKS=4 banks form one super-tile
  [128, 4, 512] f32 (bank slices 2KB-aligned so each matmul stays in-bank).
- Only a 40-wide diagonal block per 32-row quarter is evacuated: rows
  32q..32q+32 need band cols 32q..32q+40 only. One copy instruction grabs
  that block across all 4 banks x 3 tiles -> [32, 4, 3, 40], so the full
  band (136 cols) never leaves PSUM: output bytes drop 136/40 = 3.4x.
- Copies alternate engines per super-tile (even -> ScalarE, odd -> VectorE)
  so the two PSUM-capable engines run in parallel and each super's bank
  recycle depends on exactly one engine.
- Input DMAs: x1 issued via GpSimd (SWDGE) and x2 via SyncE (HWDGE) to
  spread issue cost off the copy engines; 24 slices pipeline the ramp.
  Each super's output chunk is DMAed immediately (kpc=1) so the drain
  overlaps compute. A dummy ScalarE op up front absorbs the one-time
  activation-table load.
- Inputs are host-cast to bf16 with the 1/C scale folded into x1 (exact:
  power of two); the host extracts the 9 diagonals from the gram blocks
  (numpy gather) and applies the OOB zero mask.
"""

import os
import numpy as np

import concourse.bass as bass
import concourse.bacc as bacc
import concourse.mybir as mybir
import concourse.tile as tile
from concourse import bass_utils

B, C, H, W = 8, 128, 96, 320
S = H * W            # 30720 flattened positions per batch element
MAXD = 4
ND = 2 * MAXD + 1    # 9 displacement channels
TP = 128             # positions per tile (PSUM partition dim)
NT = S // TP         # 240 tiles
NB = TP + 2 * MAXD   # 136 band columns per tile
GRP = 3              # matmul outputs packed per psum bank (3*136*4B < 2KB)
BANKS = 4            # psum banks per super-tile (copies span them)
NSUP = NT // (GRP * BANKS)  # 20 super-tiles
R = 32               # rows per diagonal block (quarter tile)
NQ = TP // R         # 4 blocks per tile
RB = R + 2 * MAXD    # 40 band cols per block
KPC = 1              # supers per output chunk (per engine)
NCH = NSUP // 2 // KPC      # 10 chunks per engine
COLS = KPC * BANKS * GRP * RB  # 480 cols per chunk per engine
NSLICE = 24          # input DMA slices per tensor
SLICE = S // NSLICE  # 1280 positions

F32 = mybir.dt.float32
BF16 = mybir.dt.bfloat16


def _build_nc(loops: int = 1):
    nc = bacc.Bacc(debug=False)
    x1 = nc.dram_tensor("x1", [C, S], BF16, kind="ExternalInput")
    # x2 is host-padded with a zero halo of MAXD on both ends: [C, S + 8];
    # dram/sbuf col j = position j - MAXD.
    x2 = nc.dram_tensor("x2", [C, S + 2 * MAXD], BF16, kind="ExternalInput")
    # gram[i, p, :480]  = ACT chunks (even supers), [i, p, 480:960] = DVE
    # chunks (odd supers); cols decode as (bk, g, n) with n the band col
    # offset within the row block: band col = 32*(p//32) + n.
    gram = nc.dram_tensor("gram", [NCH, TP, 2 * COLS], BF16,
                          kind="ExternalOutput")

    with tile.TileContext(nc) as tc:
        with (
            tc.tile_pool(name="x1p", bufs=1) as x1p,
            tc.tile_pool(name="x2p", bufs=1) as x2p,
            tc.tile_pool(name="psp", bufs=2, space="PSUM") as psp,
            tc.tile_pool(name="outp", bufs=2 * NCH) as outp,
        ):
            x1full = x1p.tile([C, S], BF16)
            x2full = x2p.tile([C, S + 2 * MAXD], BF16)
            warm = x1p.tile([1, 8], BF16, name="warm")
            nc.vector.memset(warm[:], 0.0)
            nc.scalar.copy(warm[:], warm[:])
            for rep in range(loops):
                for i in range(NSLICE):
                    lo, hi = i * SLICE, (i + 1) * SLICE
                    nc.gpsimd.dma_start(out=x1full[:, lo:hi], in_=x1[:, lo:hi])
                    xhi = hi + 2 * MAXD if i == NSLICE - 1 else hi
                    nc.sync.dma_start(out=x2full[:, lo:xhi], in_=x2[:, lo:xhi])

                for sp in range(NSUP):
                    e = sp % 2
                    i = sp // 2  # chunk index for this engine (KPC=1)
                    ot = outp.tile([TP, BANKS, GRP, RB], BF16,
                                   name=f"ot{rep}_{sp}", tag=f"ot{e}")
                    # [TP, BANKS, 512] f32: each bank slice is exactly one
                    # 2KB PSUM bank, so every matmul output stays in-bank.
                    ps = psp.tile([TP, BANKS, 512], F32)
                    for bk in range(BANKS):
                        for g in range(GRP):
                            t = (sp * BANKS + bk) * GRP + g
                            nc.tensor.matmul(
                                ps[:, bk, NB * g : NB * (g + 1)],
                                lhsT=x1full[:, TP * t : TP * (t + 1)],
                                rhs=x2full[:, TP * t : TP * t + NB],
                                start=True,
                                stop=True,
                            )
                    cp = nc.scalar.copy if e == 0 else nc.vector.tensor_copy
                    psb = ps[:, :, : GRP * NB].rearrange(
                        "p b (g n) -> p b g n", n=NB
                    )
                    for q in range(NQ):
                        cp(
                            ot[R * q : R * (q + 1)],
                            psb[R * q : R * (q + 1), :, :, R * q : R * q + RB],
                        )
                    nc.sync.dma_start(
                        out=gram[i, :, e * COLS : (e + 1) * COLS], in_=ot[:]
                    )
    nc.compile()
    return nc


_NC_CACHE = {}


def _get_nc(loops: int = 1):
    key = f"nc{loops}"
    if key not in _NC_CACHE:
        _NC_CACHE[key] = _build_nc(loops)
    return _NC_CACHE[key]


# host-side diagonal gather: the block for rows 32q..32q+32 holds band cols
# 32q..32q+40; out[j] for row p lives at n = (p % 32) + j.
_N_IDX = (np.arange(TP) % R)[:, None] + np.arange(ND)[None, :]  # [128, 9]


def _extract(gram: np.ndarray) -> np.ndarray:
    """gram [NCH, 128, 960] -> out [ND, H, W] (OOB masked)."""
    g6 = gram.reshape(NCH, TP, 2, BANKS, GRP, RB)
    idx = _N_IDX[None, :, None, None, None, :]
    sel = np.take_along_axis(g6, idx, axis=5)  # [NCH, 128, 2, BANKS, GRP, 9]
    # tile t = ((2i + e) * BANKS + bk) * GRP + g -> axis order (i, e, bk, g)
    band = sel.transpose(0, 2, 3, 4, 1, 5).reshape(NT, TP, ND)
    out = band.astype(np.float32).transpose(2, 0, 1).reshape(ND, H, W)
    out = np.ascontiguousarray(out)
    for j in range(ND):
        d = j - MAXD
        if d < 0:
            out[j, :, :-d] = 0.0
        elif d > 0:
            out[j, :, W - d :] = 0.0
    return out


def kernel(x1: np.ndarray, x2: np.ndarray) -> np.ndarray:
    assert x1.shape == (B, C, H, W) and x2.shape == (B, C, H, W)
    import ml_dtypes

    bf16 = ml_dtypes.bfloat16
    nc = _get_nc()
    # fold the 1/C mean scale into x1 (C = 128: exact exponent shift in bf16)
    x1b = (x1.reshape(B, C, S) * np.float32(1.0 / C)).astype(bf16)
    x2p = np.zeros((B, C, S + 2 * MAXD), dtype=bf16)
    x2p[:, :, MAXD : MAXD + S] = x2.reshape(B, C, S).astype(bf16)
    in_maps = [{"x1": np.ascontiguousarray(x1b[b]), "x2": x2p[b]} for b in range(B)]

    res = bass_utils.run_bass_kernel_spmd(
        nc, in_maps, core_ids=list(range(B)), trace=False
    )
    _NC_CACHE["last_results"] = res
    out = np.stack([_extract(res.results[b]["gram"]) for b in range(B)], axis=0)
    return out.astype(np.float32)
